# revision 3
# baseline (speedup 1.0000x reference)
"""BitLinear (ternary-weight linear) kernel for Trainium2, 8 NeuronCores.

Computation:  out = x @ (w_ternary * scale)^T
  where scale = max(mean(|weight|), 1e-5)
        w_ternary = clip(round(weight / scale), -1, 1)  in {-1, 0, 1}

Strategy (v2):
  - Host: quantize the 4 MB weight, pre-transpose it to wT [in, out] in
    bf16 (ternary values are exact in bf16), and pre-transpose each
    core's x slice to xT [in, s] in bf16.  bf16 x rounding gives
    ~1.5e-3 max-rel output error (tolerance 2e-2); the fp32 scale is
    applied by the scalar engine during the PSUM->SBUF output copy.
  - Device (data-parallel over batch, 1 batch row per core): pure GEMM,
    no on-chip transposes.  Per 512-column s-chunk: DMA the 8 k-tiles
    of xT (1 KB/partition lines), then for each of the 4 s-blocks run
    2 PSUM halves x 8 accumulating bf16 matmuls (lhsT = xT slice
    [128 i, 128 s] stationary, rhs = wT slice [128 i, 512 o] moving),
    scalar-copy with scale, DMA store.  bf16 matmuls issue at ~216 ns
    (FWL hides LDWEIGHTS); PE is the bottleneck at ~221 us.
"""

import numpy as np

B, S, IN, OUT = 8, 8192, 1024, 1024
N_CORES = 8
P = 128
SC = 512                 # s-chunk width
N_CHUNKS = S // SC       # 16
BLOCKS_PER_CHUNK = SC // P  # 4
K_TILES = IN // P        # 8
EPS = 1e-5

_compiled = None


def _build():
    import concourse.bacc as bacc
    import concourse.mybir as mybir
    import concourse.tile as tile

    BF = mybir.dt.bfloat16
    F32 = mybir.dt.float32

    nc = bacc.Bacc()
    xt = nc.declare_dram_parameter("xt", [IN, S], BF, isOutput=False)
    wt = nc.declare_dram_parameter("wt", [IN, OUT], BF, isOutput=False)
    scale_t = nc.declare_dram_parameter("scale", [1, 1], F32, isOutput=False)
    out = nc.declare_dram_parameter("out", [S, OUT], F32, isOutput=True)

    with tile.TileContext(nc) as tc:
        with (
            tc.tile_pool(name="const", bufs=1) as constp,
            tc.tile_pool(name="xtp", bufs=4) as xtp,
            tc.tile_pool(name="outp", bufs=4) as outp,
            tc.tile_pool(name="pso", bufs=8, space="PSUM") as pso,
        ):
            xt_r = xt[:].rearrange("(a p) s -> p a s", p=P)
            wt_r = wt[:].rearrange("(a p) o -> p a o", p=P)

            xt_tiles = {}

            def load_chunk(c, split=False):
                if c < N_CHUNKS and c not in xt_tiles:
                    t = xtp.tile([P, K_TILES, SC], BF, tag="xt",
                                 name=f"xt_{c}")
                    if split:
                        # per-k DMAs so the first matmuls' operand waits
                        # release progressively during startup
                        for k in range(K_TILES):
                            nc.sync.dma_start(
                                out=t[:, k, :],
                                in_=xt_r[:, k, c * SC:(c + 1) * SC],
                            )
                    else:
                        nc.sync.dma_start(
                            out=t, in_=xt_r[:, :, c * SC:(c + 1) * SC]
                        )
                    xt_tiles[c] = t

            # Two HWDGE rings in parallel at startup: sync (SP ring)
            # carries the x chunks, scalar (Act ring) carries the weight
            # in h-half k-slices ordered so the h=0 matmuls of the first
            # blocks see their operands earliest.  Stores later also go
            # on the Act ring, so they never queue behind x loads.
            wt_sb = constp.tile([P, K_TILES, OUT], BF)
            load_chunk(0, split=True)
            for h in range(2):
                for k in range(K_TILES):
                    nc.scalar.dma_start(
                        out=wt_sb[:, k, h * 512:(h + 1) * 512],
                        in_=wt_r[:, k, h * 512:(h + 1) * 512],
                    )
            load_chunk(1)
            load_chunk(2)

            scale_sb = constp.tile([P, 1], F32)
            nc.gpsimd.dma_start(
                out=scale_sb, in_=scale_t[:].to_broadcast((P, 1))
            )

            for c in range(N_CHUNKS):
                xt_sb = xt_tiles.pop(c)
                load_chunk(c + 3)
                for sb in range(BLOCKS_PER_CHUNK):
                    lo_s = sb * P
                    out_sb = outp.tile([P, OUT], F32)
                    s0 = c * SC + sb * P
                    last = (c == N_CHUNKS - 1 and sb == BLOCKS_PER_CHUNK - 1)
                    for h in range(2):
                        po_h = pso.tile([P, 512], F32, tag="pso",
                                        name=f"po{c}_{sb}_{h}")
                        for k in range(K_TILES):
                            nc.tensor.matmul(
                                po_h,
                                lhsT=xt_sb[:, k, lo_s:lo_s + P],
                                rhs=wt_sb[:, k, h * 512:(h + 1) * 512],
                                start=(k == 0),
                                stop=(k == K_TILES - 1),
                            )
                        # last block: 256-wide pieces, store each as soon
                        # as copied, to shorten the closing chain
                        n_pieces = 2 if last else 1
                        cw = 512 // n_pieces
                        for cc in range(n_pieces):
                            lo = h * 512 + cc * cw
                            nc.scalar.activation(
                                out_sb[:, lo:lo + cw],
                                po_h[:, cc * cw:(cc + 1) * cw],
                                mybir.ActivationFunctionType.Copy,
                                scale=scale_sb[:, 0:1],
                            )
                            if last:
                                nc.scalar.dma_start(
                                    out=out[s0:s0 + P, lo:lo + cw],
                                    in_=out_sb[:, lo:lo + cw],
                                )
                    if not last:
                        nc.scalar.dma_start(
                            out=out[s0:s0 + P, :], in_=out_sb
                        )
    nc.finalize()
    return nc


def _get_compiled():
    global _compiled
    if _compiled is None:
        _compiled = _build()
    return _compiled


def quantize_host(weight: np.ndarray):
    """Mirror of the reference ste_quantize, done on host in fp32.

    The mean is computed in float64 then rounded to fp32 so it tracks the
    true mean more closely than any fp32 summation order.
    """
    scale = np.float32(max(np.mean(np.abs(weight), dtype=np.float64), EPS))
    w_t = np.clip(np.round(weight / scale), -1.0, 1.0).astype(np.float32)
    return w_t, scale


def make_in_maps(x: np.ndarray, weight: np.ndarray):
    import ml_dtypes

    w_t, scale = quantize_host(weight)
    wt_T = np.ascontiguousarray(w_t.T).astype(ml_dtypes.bfloat16)  # [in, out]
    scale_arr = np.array([[scale]], dtype=np.float32)
    # per-core transposed bf16 activations [in, s]
    xbt = np.ascontiguousarray(x.transpose(0, 2, 1)).astype(ml_dtypes.bfloat16)
    return [
        {"xt": xbt[c], "wt": wt_T, "scale": scale_arr}
        for c in range(N_CORES)
    ], scale


def kernel(x: np.ndarray, weight: np.ndarray) -> np.ndarray:
    from concourse.bass_utils import run_bass_kernel_spmd

    x = np.asarray(x, dtype=np.float32)
    weight = np.asarray(weight, dtype=np.float32)
    assert x.shape == (B, S, IN) and weight.shape == (OUT, IN)
    in_maps, _ = make_in_maps(x, weight)
    nc = _get_compiled()
    res = run_bass_kernel_spmd(nc, in_maps, core_ids=list(range(N_CORES)))
    return np.stack([res.results[c]["out"] for c in range(N_CORES)], axis=0)
